# revision 4
# baseline (speedup 1.0000x reference)
"""Trainium2 Bass kernel for nn_ConnectionG2C (graph-to-image cross-attention block).

Reference computation (per batch element b, all in fp32 in the oracle):
    g   = input_graph[b].T                          # [G=32, N=1024]
    K   = Wk @ g + bk                               # [C=256, N]
    V   = Wv @ g + bv                               # [C, N]
    Q   = Wq @ x + bq, x = image[b] as [C, P=4096]  # [C, P]
    att = softmax_over_P( Q^T K / sqrt(C) )         # [P, N], softmax over P
    msg = V @ att^T                                 # [C, P]
    h   = LeakyReLU_0.1( BN( conv1x1(msg) ) )
    h2  = conv3x3(h) + b2
    out = image + conv1x1(h2) + b3

Sharding: data-parallel over batch B=8 -> one batch element per NeuronCore.

Per-core kernel strategy (all matmuls bf16, fp32 PSUM accumulation, fp32
residual add at the end so the dominant `image +` term is exact):
  - host pre-transposes/pre-casts all weights; BN is folded into conv1;
    bk/bv folded via an ones-row appended to g.
  - attention scores are computed transposed, attT[N, P], so the softmax
    axis (P) is the free dim: exp runs on the scalar engine straight out of
    PSUM with a per-instruction accumulated row sum; the 1/sum normalizer is
    folded into V (per-n scaling) instead of rescaling the whole [N, P] map.
  - 3x3 conv is 9 shifted 1x1 matmuls accumulated in PSUM, reading from a
    zero-padded [C, 66, 66] SBUF image.
"""

import os
from contextlib import ExitStack

import ml_dtypes
import numpy as np

BF16 = ml_dtypes.bfloat16

B, C, W, H, N, G = 8, 256, 64, 64, 1024, 32
P = W * H            # 4096 pixels
PC = 8               # pixel chunks of 512
FD = 512             # matmul free dim / PSUM bank
NCH = 8              # n chunks of 128
COC = 2              # channel chunks of 128

_BUILT = None


def _build_module():
    import concourse.bacc as bacc
    import concourse.mybir as mybir
    import concourse.tile as tile

    f32 = mybir.dt.float32
    bf16 = mybir.dt.bfloat16
    Alu = mybir.AluOpType
    Act = mybir.ActivationFunctionType

    nc = bacc.Bacc("TRN2", target_bir_lowering=False)

    # ---- DRAM tensors ----
    d_img = nc.dram_tensor("img", [C, P], f32, kind="ExternalInput")
    d_gx = nc.dram_tensor("gx", [128, N], bf16, kind="ExternalInput")
    d_wqt = nc.dram_tensor("wqt", [128, 2, C], bf16, kind="ExternalInput")
    d_wkt = nc.dram_tensor("wkt", [128, C], bf16, kind="ExternalInput")
    d_wvt = nc.dram_tensor("wvt", [128, C], bf16, kind="ExternalInput")
    d_a1t = nc.dram_tensor("a1t", [128, 2, C], bf16, kind="ExternalInput")
    d_w2t = nc.dram_tensor("w2t", [128, 18, C], bf16, kind="ExternalInput")
    d_w3t = nc.dram_tensor("w3t", [128, 2, C], bf16, kind="ExternalInput")
    d_bq = nc.dram_tensor("bq", [128, 2], f32, kind="ExternalInput")
    d_b1 = nc.dram_tensor("b1", [128, 2], f32, kind="ExternalInput")
    d_b2 = nc.dram_tensor("b2", [128, 2], f32, kind="ExternalInput")
    d_b3 = nc.dram_tensor("b3", [128, 2], f32, kind="ExternalInput")
    d_out = nc.dram_tensor("out", [C, P], f32, kind="ExternalOutput")

    with tile.TileContext(nc) as tc, ExitStack() as ctx:
        wpool = ctx.enter_context(tc.tile_pool(name="w", bufs=1))
        big = ctx.enter_context(tc.tile_pool(name="big", bufs=1))
        small = ctx.enter_context(tc.tile_pool(name="small", bufs=4))
        outp = ctx.enter_context(tc.tile_pool(name="outp", bufs=4))
        psum = ctx.enter_context(tc.tile_pool(name="psum", bufs=8, space="PSUM"))

        ps_count = [0]

        def ps_tile():
            ps_count[0] += 1
            return psum.tile([128, FD], f32, tag="ps", name=f"ps{ps_count[0]}")

        # ---- weight / input DMAs ----
        wqt = wpool.tile([128, 2, C], bf16, tag="wqt")
        nc.sync.dma_start(out=wqt, in_=d_wqt[:])
        wkt = wpool.tile([128, C], bf16, tag="wkt")
        nc.sync.dma_start(out=wkt, in_=d_wkt[:])
        wvt = wpool.tile([128, C], bf16, tag="wvt")
        nc.sync.dma_start(out=wvt, in_=d_wvt[:])
        a1t = wpool.tile([128, 2, C], bf16, tag="a1t")
        nc.sync.dma_start(out=a1t, in_=d_a1t[:])
        w2t = wpool.tile([128, 18, C], bf16, tag="w2t")
        nc.sync.dma_start(out=w2t, in_=d_w2t[:])
        w3t = wpool.tile([128, 2, C], bf16, tag="w3t")
        nc.sync.dma_start(out=w3t, in_=d_w3t[:])
        bq = wpool.tile([128, 2], f32, tag="bq")
        nc.sync.dma_start(out=bq, in_=d_bq[:])
        b1 = wpool.tile([128, 2], f32, tag="b1")
        nc.sync.dma_start(out=b1, in_=d_b1[:])
        b2 = wpool.tile([128, 2], f32, tag="b2")
        nc.sync.dma_start(out=b2, in_=d_b2[:])
        b3 = wpool.tile([128, 2], f32, tag="b3")
        nc.sync.dma_start(out=b3, in_=d_b3[:])
        gx = wpool.tile([128, N], bf16, tag="gx")
        nc.sync.dma_start(out=gx, in_=d_gx[:])

        img = big.tile([128, 2, P], f32, tag="img")
        for co in range(COC):
            nc.sync.dma_start(out=img[:, co, :], in_=d_img[co * 128:(co + 1) * 128, :])

        # ---- cast image to bf16 ----
        x = big.tile([128, 2, P], bf16, tag="xh2")
        for co in range(COC):
            nc.vector.tensor_copy(out=x[:, co, :], in_=img[:, co, :])

        # ---- K = Wk @ g + bk   (bias via ones-row in gx/wkt) ----
        k_sb = big.tile([128, 2, N], bf16, tag="k")
        for co in range(COC):
            for n5 in range(2):
                ps = ps_tile()
                nc.tensor.matmul(ps, lhsT=wkt[:, co * 128:(co + 1) * 128],
                                 rhs=gx[:, n5 * FD:(n5 + 1) * FD],
                                 start=True, stop=True)
                nc.vector.tensor_copy(out=k_sb[:, co, n5 * FD:(n5 + 1) * FD], in_=ps)

        # ---- V^T[n, c] = g^T @ Wv^T + bv ----
        vt = big.tile([128, NCH, C], bf16, tag="vt")
        for nch in range(NCH):
            ps = ps_tile()
            nc.tensor.matmul(ps[:, :C], lhsT=gx[:, nch * 128:(nch + 1) * 128],
                             rhs=wvt[:, :], start=True, stop=True)
            nc.vector.tensor_copy(out=vt[:, nch, :], in_=ps[:, :C])

        # ---- Q = Wq @ x + bq ----
        q = big.tile([128, 2, P], bf16, tag="qhp")
        for pch in range(PC):
            for co in range(COC):
                ps = ps_tile()
                for ci in range(COC):
                    nc.tensor.matmul(ps, lhsT=wqt[:, ci, co * 128:(co + 1) * 128],
                                     rhs=x[:, ci, pch * FD:(pch + 1) * FD],
                                     start=(ci == 0), stop=(ci == 1))
                nc.vector.tensor_scalar_add(out=q[:, co, pch * FD:(pch + 1) * FD],
                                            in0=ps, scalar1=bq[:, co:co + 1])

        # ---- attT[n, p] = exp(K^T Q / 16); row sums; fold 1/sum into V ----
        attT = big.tile([128, NCH, P], bf16, tag="attT")
        for nch in range(NCH):
            sums = small.tile([128, PC], f32, tag="sums")
            for pch in range(PC):
                ps = ps_tile()
                for ci in range(COC):
                    nc.tensor.matmul(ps, lhsT=k_sb[:, ci, nch * 128:(nch + 1) * 128],
                                     rhs=q[:, ci, pch * FD:(pch + 1) * FD],
                                     start=(ci == 0), stop=(ci == 1))
                nc.scalar.activation(out=attT[:, nch, pch * FD:(pch + 1) * FD],
                                     in_=ps, func=Act.Exp, scale=1.0 / 16.0,
                                     accum_out=sums[:, pch:pch + 1])
            s = small.tile([128, 1], f32, tag="s")
            nc.vector.reduce_sum(out=s, in_=sums, axis=mybir.AxisListType.X)
            r = small.tile([128, 1], f32, tag="r")
            nc.vector.reciprocal(out=r, in_=s)
            nc.vector.tensor_scalar_mul(out=vt[:, nch, :], in0=vt[:, nch, :],
                                        scalar1=r[:, 0:1])

        # ---- msg = (V/s) @ attT ----
        msg = big.tile([128, 2, P], bf16, tag="msg")
        for pch in range(PC):
            for co in range(COC):
                ps = ps_tile()
                for nch in range(NCH):
                    nc.tensor.matmul(ps, lhsT=vt[:, nch, co * 128:(co + 1) * 128],
                                     rhs=attT[:, nch, pch * FD:(pch + 1) * FD],
                                     start=(nch == 0), stop=(nch == NCH - 1))
                nc.vector.tensor_copy(out=msg[:, co, pch * FD:(pch + 1) * FD], in_=ps)

        # ---- conv1 (BN folded) + LeakyReLU(0.1) into padded [66,66] image ----
        hpad = big.tile([128, 2, 66, 66], bf16, tag="qhp")
        for co in range(COC):
            nc.vector.memset(hpad[:, co, 0, :], 0.0)
            nc.vector.memset(hpad[:, co, 65, :], 0.0)
            nc.vector.memset(hpad[:, co, :, 0:1], 0.0)
            nc.vector.memset(hpad[:, co, :, 65:66], 0.0)
        for pch in range(PC):
            r0 = pch * 8
            for co in range(COC):
                ps = ps_tile()
                for ci in range(COC):
                    nc.tensor.matmul(ps, lhsT=a1t[:, ci, co * 128:(co + 1) * 128],
                                     rhs=msg[:, ci, pch * FD:(pch + 1) * FD],
                                     start=(ci == 0), stop=(ci == 1))
                nc.scalar.activation(out=ps, in_=ps, func=Act.Identity,
                                     bias=b1[:, co:co + 1], scale=1.0)
                psv = ps.rearrange("p (a b) -> p a b", a=8)
                dst = hpad[:, co, 1 + r0:1 + r0 + 8, 1:65]
                nc.vector.tensor_scalar_mul(out=dst, in0=psv, scalar1=0.1)
                nc.vector.tensor_tensor(out=dst, in0=psv, in1=dst, op=Alu.max)

        # ---- conv2 3x3: 9 shifted 1x1 matmuls, accumulate in PSUM ----
        h2 = big.tile([128, 2, P], bf16, tag="xh2")
        for pch in range(PC):
            r0 = pch * 8
            for co in range(COC):
                ps = ps_tile()
                idx = 0
                for ky in range(3):
                    for kx in range(3):
                        for ci in range(COC):
                            nc.tensor.matmul(
                                ps,
                                lhsT=w2t[:, (ky * 3 + kx) * 2 + ci,
                                         co * 128:(co + 1) * 128],
                                rhs=hpad[:, ci, r0 + ky:r0 + ky + 8, kx:kx + 64],
                                start=(idx == 0), stop=(idx == 17))
                            idx += 1
                nc.vector.tensor_scalar_add(out=h2[:, co, pch * FD:(pch + 1) * FD],
                                            in0=ps, scalar1=b2[:, co:co + 1])

        # ---- conv3 1x1 + bias + residual ----
        for pch in range(PC):
            for co in range(COC):
                ps = ps_tile()
                for ci in range(COC):
                    nc.tensor.matmul(ps, lhsT=w3t[:, ci, co * 128:(co + 1) * 128],
                                     rhs=h2[:, ci, pch * FD:(pch + 1) * FD],
                                     start=(ci == 0), stop=(ci == 1))
                ot = outp.tile([128, FD], f32, tag="ot")
                nc.vector.scalar_tensor_tensor(
                    out=ot, in0=ps, scalar=b3[:, co:co + 1],
                    in1=img[:, co, pch * FD:(pch + 1) * FD],
                    op0=Alu.add, op1=Alu.add)
                nc.sync.dma_start(
                    out=d_out[co * 128:(co + 1) * 128, pch * FD:(pch + 1) * FD],
                    in_=ot)

    nc.compile()
    return nc


def get_module():
    global _BUILT
    if _BUILT is None:
        _BUILT = _build_module()
    return _BUILT


def prepare_in_maps(input_graph, input_image, Wq, bq, Wk, bk, Wv, bv,
                    conv1_w, bn_gamma, bn_beta, bn_mean, bn_var,
                    conv2_w, conv2_b, conv3_w, conv3_b):
    """Host-side weight preprocessing + per-core input maps (numpy only)."""
    f32 = np.float32

    def chunked_lhsT(w_t):  # [ci=256, co=256] -> [128, 2, 256] bf16
        return np.ascontiguousarray(
            w_t.reshape(2, 128, C).transpose(1, 0, 2)).astype(BF16)

    inv = 1.0 / np.sqrt(np.asarray(bn_var, f32) + f32(1e-5))
    scale = np.asarray(bn_gamma, f32) * inv
    A1 = np.asarray(conv1_w, f32)[:, :, 0, 0] * scale[:, None]
    b1 = np.asarray(bn_beta, f32) - np.asarray(bn_mean, f32) * scale

    wqt = chunked_lhsT(np.asarray(Wq, f32).T)
    a1t = chunked_lhsT(A1.T)
    w3t = chunked_lhsT(np.asarray(conv3_w, f32)[:, :, 0, 0].T)

    # conv2 taps: [O,I,3,3] -> per tap (ky,kx) the [ci, co] transpose, chunked
    t2 = np.asarray(conv2_w, f32).transpose(2, 3, 1, 0).reshape(9, C, C)
    w2t = np.ascontiguousarray(
        t2.reshape(9, 2, 128, C).transpose(2, 0, 1, 3).reshape(128, 18, C)
    ).astype(BF16)

    wkt = np.zeros((128, C), f32)
    wkt[:G] = np.asarray(Wk, f32).T
    wkt[G] = np.asarray(bk, f32)
    wvt = np.zeros((128, C), f32)
    wvt[:G] = np.asarray(Wv, f32).T
    wvt[G] = np.asarray(bv, f32)

    def per_chunk_bias(v):  # [256] -> [128, 2] f32
        return np.ascontiguousarray(np.asarray(v, f32).reshape(2, 128).T)

    shared = {
        "wqt": wqt, "wkt": wkt.astype(BF16), "wvt": wvt.astype(BF16),
        "a1t": a1t, "w2t": w2t, "w3t": w3t,
        "bq": per_chunk_bias(bq), "b1": per_chunk_bias(b1),
        "b2": per_chunk_bias(conv2_b), "b3": per_chunk_bias(conv3_b),
    }

    graph = np.asarray(input_graph, f32)
    image = np.asarray(input_image, f32)
    in_maps = []
    for b in range(B):
        gx = np.zeros((128, N), f32)
        gx[:G] = graph[b].T
        gx[G] = 1.0
        m = dict(shared)
        m["gx"] = gx.astype(BF16)
        m["img"] = np.ascontiguousarray(image[b].reshape(C, P))
        in_maps.append(m)
    return in_maps


def run(inputs, trace=False, trace_kwargs=None):
    from concourse.bass_utils import run_bass_kernel_spmd

    nc = get_module()
    in_maps = prepare_in_maps(**inputs)
    res = run_bass_kernel_spmd(
        nc, in_maps, core_ids=list(range(B)), trace=trace,
        **(trace_kwargs or {}))
    out = np.stack([r["out"] for r in res.results]).reshape(B, C, W, H)
    return out, res


def kernel(**inputs):
    out, _ = run(inputs, trace=False)
    return out
